# revision 1
# baseline (speedup 1.0000x reference)
"""Cubic B-spline FFD 3D upsampling kernel for Trainium2 (8 NeuronCores).

Reference computation: v [4,3,44,52,44] f32 -> out [4,3,160,192,160] f32 via three
separable stride-4 transposed convs (cubic B-spline, kernel len 15) + crop [4:4+D].

Sharding: output z axis (160) split into 8 chunks of 20; core c consumes input
z-planes [5c, 5c+8) (no halo exchange needed) and writes its own [12,20,192,160]
block. Host slices/concats only (no transposes).

Per-core pipeline (all shapes [partition, free]):
  L0 [128=(g2*64 + yi52), (b6, zi8, xi44)]      bf16, DMA-in
  z-pass on DVE: fused scalar_tensor_tensor MACs (polyphase, zo=4k+r)
  L1 [128, (b6, zo20, xi-pad64)]                bf16
  y-pass on PE:  out[xi,yo] = L1[yi,xi].T @ Wy[yi,yo]  per (g,b,zo), bf16
  L2b [128=(g2*64 + xi44pad), (zo20, yo192)]    bf16  (per b)
  x-pass on PE:  out[m,xo] = L2b[xi, m-chunk].T @ Wx[xi,xo],  m=(zo,yo) flat
  PSUM f32 -> SBUF f32 -> DMA out, xo contiguous (final layout, no transpose)
"""

import numpy as np

N_CORES = 8
ZIN, YIN, XIN = 44, 52, 44
ZOUT, YOUT, XOUT = 160, 192, 160
BC = 12  # batch*channels
ZSH = ZOUT // N_CORES      # 20 output z per core
ZISH = 8                   # input z planes per core


def _bspline_kernel():
    x = (np.arange(15) - 7) / 4.0
    t = np.abs(x)
    return np.where(
        t < 1.0, 2.0 / 3.0 + (0.5 * t - 1.0) * t**2,
        np.where(t < 2.0, ((2.0 - t) ** 3) / 6.0, 0.0)
    ).astype(np.float32)


_W = _bspline_kernel()


def _exp_mat(n_in, n_out):
    """M[i, o] = weight of control point i on (post-crop) output o."""
    M = np.zeros((n_in, n_out), dtype=np.float32)
    for o in range(n_out):
        ilo = int(np.ceil((o - 3) / 4))
        ihi = (o + 11) // 4
        for i in range(max(ilo, 0), min(ihi, n_in - 1) + 1):
            n = 4 * i - o + 3
            if 0 <= n < 15:
                M[i, o] = _W[n]
    return M


def _ztaps():
    """Per phase r: list of (tap t, weight) with input plane = k + t for zo=4k+r."""
    out = []
    for r in range(4):
        taps = []
        for t in range(4):
            n = 4 * t + 3 - r
            if 0 <= n < 15:
                taps.append((t, float(_W[n])))
        out.append(taps)
    return out


_NC_CACHE = {}


def _build_nc():
    import concourse.bacc as bacc
    import concourse.mybir as mybir
    from concourse.tile import TileContext

    FP32 = mybir.dt.float32
    BF16 = mybir.dt.bfloat16
    MULT = mybir.AluOpType.mult
    ADD = mybir.AluOpType.add

    nc = bacc.Bacc()
    v = nc.declare_dram_parameter("v", [BC, ZISH, YIN, XIN], BF16, isOutput=False)
    wy = nc.declare_dram_parameter("wy", [128, YOUT], BF16, isOutput=False)
    wx = nc.declare_dram_parameter("wx", [128, XOUT], BF16, isOutput=False)
    out = nc.declare_dram_parameter(
        "out", [BC, ZSH, YOUT, XOUT], FP32, isOutput=True
    )
    outflat = out.rearrange("b z y x -> (b z y) x")  # [46080, 160]

    ztaps = _ztaps()
    XP = 64  # xi padded to 64 in L1 so two (g) matmuls col-tile at bases {0, 64}

    with TileContext(nc) as tc:
        with (
            tc.tile_pool(name="const", bufs=1) as cpool,
            tc.tile_pool(name="io", bufs=1) as iopool,
            tc.tile_pool(name="l2", bufs=4) as l2pool,
            tc.tile_pool(name="stage", bufs=12) as stpool,
            tc.tile_pool(name="psy", bufs=4, space="PSUM") as psy,
            tc.tile_pool(name="psx", bufs=4, space="PSUM") as psx,
        ):
            wyt = cpool.tile([128, YOUT], BF16)
            nc.sync.dma_start(out=wyt[:, :], in_=wy[:, :])
            wxt = cpool.tile([128, XOUT], BF16)
            nc.sync.dma_start(out=wxt[:, :], in_=wx[:, :])

            L0 = iopool.tile([128, 6 * ZISH * XIN], BF16)   # (b, zi, xi)
            L1 = iopool.tile([128, 6 * ZSH * XP], BF16)     # (b, zo, xi-pad)

            for g in range(2):
                nc.sync.dma_start(
                    out=L0[64 * g:64 * g + YIN, :]
                    .rearrange("p (b z x) -> p b z x", b=6, z=ZISH),
                    in_=v[6 * g:6 * g + 6].rearrange("b z y x -> y b z x"),
                )

            L0v = L0.rearrange("p (b z x) -> p b z x", b=6, z=ZISH)
            # zo = 4k + r  (k-major, r-minor view)
            L1r = L1.rearrange("p (b k r x) -> p b k r x", b=6, k=5, r=4)
            L1z = L1.rearrange("p (b z x) -> p b z x", b=6, z=ZSH)

            # ---- z-pass (DVE fused MACs), all b at once, 15 instructions ----
            for g in range(2):
                lo, hi = 64 * g, 64 * g + YIN
                for r in range(4):
                    dst = L1r[lo:hi, :, :, r, 0:XIN]
                    t0, w0 = ztaps[r][0]
                    nc.vector.tensor_scalar_mul(dst, L0v[lo:hi, :, t0:t0 + 5, :], w0)
                    for t, w in ztaps[r][1:]:
                        nc.vector.scalar_tensor_tensor(
                            out=dst, in0=L0v[lo:hi, :, t:t + 5, :], scalar=w,
                            in1=dst, op0=MULT, op1=ADD,
                        )

            # ---- per-b: y-pass (PE) -> L2b, then x-pass (PE) -> DMA out ----
            ncopy = 0
            for b in range(6):
                L2b = l2pool.tile([128, ZSH * YOUT], BF16)
                for zp in range(ZSH // 2):
                    py = psy.tile([128, 2 * YOUT], FP32)
                    for i in range(2):
                        zo = 2 * zp + i
                        for g in range(2):
                            nc.tensor.matmul(
                                py[64 * g:64 * g + XP, i * YOUT:(i + 1) * YOUT],
                                lhsT=L1z[64 * g:64 * g + YIN, b, zo, :],
                                rhs=wyt[64 * g:64 * g + YIN, :],
                                start=True, stop=True,
                            )
                    dst = L2b[:, zp * 2 * YOUT:(zp + 1) * 2 * YOUT]
                    if ncopy % 2 == 0:
                        nc.vector.tensor_copy(out=dst, in_=py[:, :])
                    else:
                        nc.scalar.copy(dst, py[:, :])
                    ncopy += 1

                for g in range(2):
                    for cg in range(10):
                        px = psx.tile([128, 3 * XOUT], FP32)
                        for j in range(3):
                            c = cg * 3 + j
                            nc.tensor.matmul(
                                px[:, j * XOUT:(j + 1) * XOUT],
                                lhsT=L2b[64 * g:64 * g + XIN,
                                         c * 128:(c + 1) * 128],
                                rhs=wxt[64 * g:64 * g + XIN, :],
                                start=True, stop=True,
                            )
                        st = stpool.tile([128, 3 * XOUT], FP32)
                        if ncopy % 2 == 0:
                            nc.vector.tensor_copy(out=st[:, :], in_=px[:, :])
                        else:
                            nc.scalar.copy(st[:, :], px[:, :])
                        ncopy += 1
                        base = (g * 6 + b) * ZSH * YOUT + cg * 384
                        nc.sync.dma_start(
                            out=outflat[base:base + 384, :].rearrange(
                                "(j p) x -> p j x", p=128),
                            in_=st.rearrange("p (j x) -> p j x", j=3),
                        )
    nc.compile()
    return nc


def _get_nc():
    if "nc" not in _NC_CACHE:
        _NC_CACHE["nc"] = _build_nc()
    return _NC_CACHE["nc"]


def kernel(v):
    import ml_dtypes
    from concourse.bass_utils import run_bass_kernel_spmd

    bf16 = ml_dtypes.bfloat16
    v = np.asarray(v).astype(np.float32).reshape(BC, ZIN, YIN, XIN)

    wy128 = np.zeros((128, YOUT), dtype=np.float32)
    wy128[0:YIN_Y] = _exp_mat(YIN_Y, YOUT)
    wy128[64:64 + YIN_Y] = wy128[0:YIN_Y]
    wx128 = np.zeros((128, XOUT), dtype=np.float32)
    wx128[0:XIN] = _exp_mat(XIN, XOUT)
    wx128[64:64 + XIN] = wx128[0:XIN]
    wy_b = wy128.astype(bf16)
    wx_b = wx128.astype(bf16)

    in_maps = []
    for c in range(N_CORES):
        slab = np.ascontiguousarray(v[:, 5 * c:5 * c + ZISH]).astype(bf16)
        in_maps.append({"v": slab, "wy": wy_b, "wx": wx_b})

    nc = _get_nc()
    res = run_bass_kernel_spmd(nc, in_maps, core_ids=list(range(N_CORES)))

    out = np.empty((BC, ZOUT, YOUT, XOUT), dtype=np.float32)
    for c in range(N_CORES):
        out[:, ZSH * c:ZSH * (c + 1)] = res.results[c]["out"]
    return out.reshape(4, 3, ZOUT, YOUT, XOUT)


YIN_Y = YIN  # y-axis input size (52)



# revision 8
# speedup vs baseline: 1.4009x; 1.4009x over previous
"""Cubic B-spline FFD 3D upsampling kernel for Trainium2 (8 NeuronCores).

Reference: v [4,3,44,52,44] f32 -> out [4,3,160,192,160] f32 via three separable
stride-4 transposed convs (cubic B-spline, len 15) + crop [4:4+D].

Sharding: output z (160) split into 8 chunks of 20; core c consumes input
z-planes [5c, 5c+8) and writes out[:, :, 20c:20c+20].

Per-core pipeline (fp16 activations, f32 PSUM):
  L0 [128=(g*64 + y52), (b6, zi8, xi44)]  <- one contiguous DMA (host pre-layouts)
  z-pass (phases r=1,2,3): DVE polyphase MACs split as
     tensor_scalar_mul (4x mode) + tensor_tensor add (2x mode) -> L1
  z-phase r=0 is folded into the y-pass as PE accumulation over 3 taps with
     pre-scaled wy weights (reads L0 directly).
  y-pass: per (g, b-pair, zo): matmul lhsT=[y52,(b2,xi44)=88] rhs=wy[y,192]
     -> psum [88, 192];  4 zo per 2-bank psum tile
  y-copy: psum -> L2 [88=(b2,xi44), m=(zo20,yo192)=3840] fp16
  x-pass: chunk j: lhsT = L2[44bm:+44, m=j::30 (128)] @ wx[44,160] -> psum
  x-copy: psum [128, 2x480] -> st [128, (r30, xo160)] fp16 (partition p holds
     output rows 30p..30p+30 of the (zo,yo) raster -> 9600B contiguous lines)
  out-DMA: 2 per bc, [128, 1920/2880] -> HBM, fp16 (host upcasts to f32)

Copies are spread across DVE / Act / GPSIMD to balance engine busy time.
"""

import numpy as np

N_CORES = 8
ZIN, YIN, XIN = 44, 52, 44
ZOUT, YOUT, XOUT = 160, 192, 160
ZI = 8              # input z planes per core
ZSH = 20            # output z per core
B6 = 6              # batch-channels per partition group
M_TOT = ZSH * YOUT  # 3840 output rows per bc
XP = 64             # xi padded to 64 (PE base-partition constraint: 0/32/64)
NCH = 30            # x-pass chunks per bc (M_TOT / 128)

# --- tuning knobs ---
# z phases folded into the y-pass as PE accumulation, per group index 0..5
PE_OFFLOAD = {0: (0, 2), 1: (0, 2), 2: (0, 2), 3: (0, 2), 4: (0, 2), 5: (0, 2)}
# engines for remaining z MACs: "pool" (gpsimd fused) or "dve" (mul+add split)
Z_ENGINE = "pool"
# weighted engine choice for PSUM->SBUF copies: (dve, act)
COPY_W = (52.0, 48.0)


def _bspline_kernel():
    x = (np.arange(15) - 7) / 4.0
    t = np.abs(x)
    return np.where(
        t < 1.0, 2.0 / 3.0 + (0.5 * t - 1.0) * t**2,
        np.where(t < 2.0, ((2.0 - t) ** 3) / 6.0, 0.0)
    ).astype(np.float32)


_W = _bspline_kernel()


def _exp_mat(n_in, n_out):
    """M[i, o] = weight of control point i on (post-crop) output o."""
    M = np.zeros((n_in, n_out), dtype=np.float32)
    for o in range(n_out):
        ilo = int(np.ceil((o - 3) / 4))
        ihi = (o + 11) // 4
        for i in range(max(ilo, 0), min(ihi, n_in - 1) + 1):
            n = 4 * i - o + 3
            if 0 <= n < 15:
                M[i, o] = _W[n]
    return M


def _ztaps():
    """Per phase r: list of (tap t, weight); input plane = k + t for zo=4k+r."""
    out = []
    for r in range(4):
        taps = []
        for t in range(4):
            n = 4 * t + 3 - r
            if 0 <= n < 15:
                taps.append((t, float(_W[n])))
        out.append(taps)
    return out


_ZTAPS = _ztaps()
_NC_CACHE = {}


def _build_nc():
    import concourse.bacc as bacc
    import concourse.mybir as mybir
    from concourse.tile import TileContext

    FP32 = mybir.dt.float32
    FP16 = mybir.dt.float16
    ADD = mybir.AluOpType.add
    MULT = mybir.AluOpType.mult

    nc = bacc.Bacc()
    v = nc.declare_dram_parameter("v", [128, B6 * ZI * XP], FP16, isOutput=False)
    wy = nc.declare_dram_parameter("wy", [128, YOUT], FP16, isOutput=False)
    n0 = len(_ZTAPS[0])
    wy0 = nc.declare_dram_parameter("wy0", [128, n0 * YOUT], FP16, isOutput=False)
    n2 = len(_ZTAPS[2])
    wy2 = nc.declare_dram_parameter("wy2", [128, n2 * YOUT], FP16, isOutput=False)
    wx = nc.declare_dram_parameter("wx", [128, XOUT], FP16, isOutput=False)
    out = nc.declare_dram_parameter("out", [12, M_TOT, XOUT], FP16, isOutput=True)

    with TileContext(nc) as tc:
        with (
            tc.tile_pool(name="const", bufs=1) as cpool,
            tc.tile_pool(name="io", bufs=1) as iopool,
            tc.tile_pool(name="tmp", bufs=4) as tmppool,
            tc.tile_pool(name="l2", bufs=3) as l2pool,
            tc.tile_pool(name="stp", bufs=3) as stpool,
            tc.tile_pool(name="psy", bufs=2, space="PSUM") as psyp,
            tc.tile_pool(name="psx", bufs=2, space="PSUM") as psxp,
        ):
            wyt = cpool.tile([128, YOUT], FP16)
            nc.sync.dma_start(out=wyt[:, :], in_=wy[:, :])
            wy0t = cpool.tile([128, n0 * YOUT], FP16)
            nc.sync.dma_start(out=wy0t[:, :], in_=wy0[:, :])
            wy0v = wy0t.rearrange("p (t y) -> p t y", t=n0)
            wy2t = cpool.tile([128, n2 * YOUT], FP16)
            nc.sync.dma_start(out=wy2t[:, :], in_=wy2[:, :])
            wy2v = wy2t.rearrange("p (t y) -> p t y", t=n2)
            wxt = cpool.tile([128, XOUT], FP16)
            nc.sync.dma_start(out=wxt[:, :], in_=wx[:, :])

            L0 = iopool.tile([128, B6 * ZI * XP], FP16)
            for g in range(2):
                nc.sync.dma_start(out=L0[64 * g:64 * g + 64, :],
                                  in_=v[64 * g:64 * g + 64, :])
            L0v = L0.rearrange("p (z b x) -> p z b x", z=ZI, b=B6)

            L1 = iopool.tile([128, B6 * ZSH * XP], FP16)
            L1v = L1.rearrange("p (k r b x) -> p k r b x", k=5, r=4, b=B6)

            groups = [(g, bp) for g in range(2) for bp in range(3)]

            xw = list(COPY_W)
            xacc = [0.0, 0.0]

            def pick():
                tot = sum(xw)
                for k in range(2):
                    xacc[k] += xw[k] / tot
                i = max(range(2), key=lambda k: xacc[k])
                xacc[i] -= 1.0
                return i

            def copy_ps(dst, src):
                if pick() == 0:
                    nc.vector.tensor_copy(out=dst, in_=src)
                else:
                    nc.scalar.copy(dst, src)

            def emit_z(gi):
                """z-pass phases r not PE-offloaded, for group index gi."""
                g, bp = groups[gi]
                lo, hi = 64 * g, 64 * g + YIN
                for r in range(4):
                    if r in PE_OFFLOAD[gi]:
                        continue
                    dst = L1v[lo:hi, :, r, 2 * bp:2 * bp + 2, 0:XIN]
                    taps = _ZTAPS[r]
                    t0, w0 = taps[0]
                    if Z_ENGINE == "pool":
                        # DVE does scaled muls (4x mode) into contiguous tmps;
                        # gpsimd (3D-AP limit, so per-b) accumulates into L1.
                        tms = []
                        for t, w in taps:
                            tm = tmppool.tile([128, 2 * 5 * XIN], FP16)
                            tmv = tm.rearrange("p (k b x) -> p k b x", k=5, b=2)
                            nc.vector.tensor_scalar_mul(
                                tmv[lo:hi], L0v[lo:hi, t:t + 5, 2 * bp:2 * bp + 2, 0:XIN], w)
                            tms.append(tmv)
                        for b in range(2):
                            dstb = L1v[lo:hi, :, r, 2 * bp + b, 0:XIN]
                            nc.gpsimd.tensor_tensor(
                                out=dstb, in0=tms[0][lo:hi, :, b, :],
                                in1=tms[1][lo:hi, :, b, :], op=ADD)
                            for tmv2 in tms[2:]:
                                nc.gpsimd.tensor_tensor(
                                    out=dstb, in0=dstb,
                                    in1=tmv2[lo:hi, :, b, :], op=ADD)
                    else:
                        nc.vector.tensor_scalar_mul(
                            dst, L0v[lo:hi, t0:t0 + 5, 2 * bp:2 * bp + 2, 0:XIN], w0)
                        for t, w in taps[1:]:
                            tm = tmppool.tile([128, 2 * 5 * XIN], FP16)
                            tmv = tm.rearrange("p (k b x) -> p k b x", k=5, b=2)
                            nc.vector.tensor_scalar_mul(
                                tmv[lo:hi], L0v[lo:hi, t:t + 5, 2 * bp:2 * bp + 2, 0:XIN], w)
                            nc.vector.tensor_tensor(
                                out=dst, in0=dst, in1=tmv[lo:hi], op=ADD)

            def emit_y(gi):
                """y-pass for group: 5 psum quads of 4 zo each -> L2 tile."""
                g, bp = groups[gi]
                lo, hi = 64 * g, 64 * g + YIN
                L2g = l2pool.tile([128, M_TOT], FP16)
                offs = (0, 192, 512, 704)
                wyo = {0: wy0v, 2: wy2v}
                for q in range(5):
                    psy = psyp.tile([128, 1024], FP32)
                    for s in range(4):
                        zo, r, off = 4 * q + s, s, offs[s]
                        if r in PE_OFFLOAD[gi]:
                            taps = _ZTAPS[r]
                            for i, (t, w) in enumerate(taps):
                                nc.tensor.matmul(
                                    psy[:, off:off + YOUT],
                                    lhsT=L0v[lo:hi, q + t, 2 * bp:2 * bp + 2, :],
                                    rhs=wyo[r][lo:hi, i, :],
                                    start=(i == 0), stop=(i == len(taps) - 1),
                                )
                        else:
                            nc.tensor.matmul(
                                psy[:, off:off + YOUT],
                                lhsT=L1v[lo:hi, q, r, 2 * bp:2 * bp + 2, :],
                                rhs=wyt[lo:hi, :],
                                start=True, stop=True,
                            )
                    psyv = psy.rearrange("p (h x) -> p h x", h=2)
                    copy_ps(
                        L2g.rearrange("p (h x) -> p h x", h=10)[:, 2 * q:2 * q + 2, :],
                        psyv[:, :, 0:2 * YOUT])
                return L2g

            def emit_x(gi, L2g):
                """x-pass + staging + out DMA for the 2 bc of this group."""
                g, bp = groups[gi]
                L2j = L2g.rearrange("p (k j) -> p k j", j=NCH)
                for bm in range(2):
                    bc = 6 * g + 2 * bp + bm
                    st = stpool.tile([128, NCH * XOUT], FP16)
                    for pair in range(5):
                        psx = psxp.tile([128, 1024], FP32)
                        for u in range(6):
                            j = 6 * pair + u
                            off = 512 * (u // 3) + 160 * (u % 3)
                            nc.tensor.matmul(
                                psx[:, off:off + XOUT],
                                lhsT=L2j[64 * bm:64 * bm + XIN, :, j],
                                rhs=wxt[64 * bm:64 * bm + XIN, :],
                                start=True, stop=True,
                            )
                        psxv = psx.rearrange("p (h x) -> p h x", h=2)
                        dst = st.rearrange("p (pr x) -> p pr x", pr=5)[:, pair, :]
                        dstv = dst.rearrange("p (h x) -> p h x", h=2)
                        copy_ps(dstv, psxv[:, :, 0:480])
                        if pair == 1:
                            nc.sync.dma_start(
                                out=out[bc].rearrange("(p r) x -> p (r x)", p=128)[:, 0:1920],
                                in_=st[:, 0:1920])
                    nc.sync.dma_start(
                        out=out[bc].rearrange("(p r) x -> p (r x)", p=128)[:, 1920:4800],
                        in_=st[:, 1920:4800])

            # --- software-pipelined emission ---
            emit_z(0)
            emit_z(1)
            L2s = {0: emit_y(0)}
            for k in range(1, 7):
                if k < 6:
                    if k + 1 < 6:
                        emit_z(k + 1)
                    L2s[k] = emit_y(k)
                emit_x(k - 1, L2s.pop(k - 1))
    nc.compile()
    return nc


def _get_nc():
    if "nc" not in _NC_CACHE:
        _NC_CACHE["nc"] = _build_nc()
    return _NC_CACHE["nc"]


def _host_weights():
    f16 = np.float16
    ey = _exp_mat(YIN, YOUT)
    ex = _exp_mat(XIN, XOUT)
    wy128 = np.zeros((128, YOUT), dtype=np.float32)
    wy128[0:YIN] = ey
    wy128[64:64 + YIN] = ey
    wx128 = np.zeros((128, XOUT), dtype=np.float32)
    wx128[0:XIN] = ex
    wx128[64:64 + XIN] = ex
    def scaled(r):
        taps = _ZTAPS[r]
        m = np.zeros((128, len(taps) * YOUT), dtype=np.float32)
        for i, (t, w) in enumerate(taps):
            m[:, i * YOUT:(i + 1) * YOUT] = wy128 * w
        return m.astype(f16)
    return wy128.astype(f16), scaled(0), scaled(2), wx128.astype(f16)


def kernel(v):
    from concourse.bass_utils import run_bass_kernel_spmd

    f16 = np.float16
    v = np.asarray(v).astype(np.float32).reshape(12, ZIN, YIN, XIN)
    wy_h, wy0_h, wy2_h, wx_h = _host_weights()

    in_maps = []
    for c in range(N_CORES):
        slab = v[:, 5 * c:5 * c + ZI]                      # [12, 8, 52, 44]
        arr = np.zeros((128, ZI, B6, XP), dtype=f16)
        arr[0:YIN, :, :, 0:XIN] = slab[0:6].transpose(2, 1, 0, 3)   # y, z, b, x
        arr[64:64 + YIN, :, :, 0:XIN] = slab[6:12].transpose(2, 1, 0, 3)
        in_maps.append({
            "v": np.ascontiguousarray(arr.reshape(128, B6 * ZI * XP)),
            "wy": wy_h, "wy0": wy0_h, "wy2": wy2_h, "wx": wx_h,
        })

    nc = _get_nc()
    res = run_bass_kernel_spmd(nc, in_maps, core_ids=list(range(N_CORES)))

    outf = np.empty((12, ZOUT, YOUT, XOUT), dtype=np.float32)
    for c in range(N_CORES):
        blk = res.results[c]["out"]                        # [12, 3840, 160] fp16
        outf[:, ZSH * c:ZSH * (c + 1)] = (
            blk.astype(np.float32).reshape(12, ZSH, YOUT, XOUT))
    return outf.reshape(4, 3, ZOUT, YOUT, XOUT)


# revision 9
# speedup vs baseline: 1.5729x; 1.1228x over previous
"""Cubic B-spline FFD 3D upsampling kernel for Trainium2 (8 NeuronCores).

Reference: v [4,3,44,52,44] f32 -> out [4,3,160,192,160] f32 via three separable
stride-4 transposed convs (cubic B-spline, len 15) + crop [4:4+D].

Sharding: output z (160) split into 8 chunks of 20; core c consumes input
z-planes [5c, 5c+8) and writes out[:, :, 20c:20c+20].

Per-core pipeline (fp16 activations, f32 PSUM):
  L0 [128=(g*64 + y52), (b6, zi8, xi44)]  <- one contiguous DMA (host pre-layouts)
  z-pass (phases r=1,2,3): DVE polyphase MACs split as
     tensor_scalar_mul (4x mode) + tensor_tensor add (2x mode) -> L1
  z-phase r=0 is folded into the y-pass as PE accumulation over 3 taps with
     pre-scaled wy weights (reads L0 directly).
  y-pass: per (g, b-pair, zo): matmul lhsT=[y52,(b2,xi44)=88] rhs=wy[y,192]
     -> psum [88, 192];  4 zo per 2-bank psum tile
  y-copy: psum -> L2 [88=(b2,xi44), m=(zo20,yo192)=3840] fp16
  x-pass: chunk j: lhsT = L2[44bm:+44, m=j::30 (128)] @ wx[44,160] -> psum
  x-copy: psum [128, 2x480] -> st [128, (r30, xo160)] fp16 (partition p holds
     output rows 30p..30p+30 of the (zo,yo) raster -> 9600B contiguous lines)
  out-DMA: 2 per bc, [128, 1920/2880] -> HBM, fp16 (host upcasts to f32)

Copies are spread across DVE / Act / GPSIMD to balance engine busy time.
"""

import numpy as np

N_CORES = 8
ZIN, YIN, XIN = 44, 52, 44
ZOUT, YOUT, XOUT = 160, 192, 160
ZI = 8              # input z planes per core
ZSH = 20            # output z per core
B6 = 6              # batch-channels per partition group
M_TOT = ZSH * YOUT  # 3840 output rows per bc
XP = 64             # xi padded to 64 (PE base-partition constraint: 0/32/64)
NCH = 30            # x-pass chunks per bc (M_TOT / 128)

# --- tuning knobs ---
# z phases folded into the y-pass as PE accumulation, per group index 0..5
PE_OFFLOAD = {0: (0, 2), 1: (0, 2), 2: (0, 2), 3: (0, 2), 4: (0, 2), 5: (0, 2)}
# per-group engine for remaining z MACs: "dve" = mul+add on DVE only,
# "hybrid" = DVE muls + gpsimd pair-adds + DVE final add
Z_MODE = {0: "dve", 1: "hybrid", 2: "hybrid", 3: "hybrid", 4: "hybrid", 5: "hybrid"}
# weighted engine choice for PSUM->SBUF copies: (dve, act)
COPY_W = (45.0, 55.0)


def _bspline_kernel():
    x = (np.arange(15) - 7) / 4.0
    t = np.abs(x)
    return np.where(
        t < 1.0, 2.0 / 3.0 + (0.5 * t - 1.0) * t**2,
        np.where(t < 2.0, ((2.0 - t) ** 3) / 6.0, 0.0)
    ).astype(np.float32)


_W = _bspline_kernel()


def _exp_mat(n_in, n_out):
    """M[i, o] = weight of control point i on (post-crop) output o."""
    M = np.zeros((n_in, n_out), dtype=np.float32)
    for o in range(n_out):
        ilo = int(np.ceil((o - 3) / 4))
        ihi = (o + 11) // 4
        for i in range(max(ilo, 0), min(ihi, n_in - 1) + 1):
            n = 4 * i - o + 3
            if 0 <= n < 15:
                M[i, o] = _W[n]
    return M


def _ztaps():
    """Per phase r: list of (tap t, weight); input plane = k + t for zo=4k+r."""
    out = []
    for r in range(4):
        taps = []
        for t in range(4):
            n = 4 * t + 3 - r
            if 0 <= n < 15:
                taps.append((t, float(_W[n])))
        out.append(taps)
    return out


_ZTAPS = _ztaps()
_NC_CACHE = {}


def _build_nc():
    import concourse.bacc as bacc
    import concourse.mybir as mybir
    from concourse.tile import TileContext

    FP32 = mybir.dt.float32
    FP16 = mybir.dt.float16
    ADD = mybir.AluOpType.add
    MULT = mybir.AluOpType.mult

    nc = bacc.Bacc()
    v = nc.declare_dram_parameter("v", [128, B6 * ZI * XP], FP16, isOutput=False)
    wy = nc.declare_dram_parameter("wy", [128, YOUT], FP16, isOutput=False)
    n0 = len(_ZTAPS[0])
    wy0 = nc.declare_dram_parameter("wy0", [128, n0 * YOUT], FP16, isOutput=False)
    n2 = len(_ZTAPS[2])
    wy2 = nc.declare_dram_parameter("wy2", [128, n2 * YOUT], FP16, isOutput=False)
    wx = nc.declare_dram_parameter("wx", [128, XOUT], FP16, isOutput=False)
    out = nc.declare_dram_parameter("out", [12, M_TOT, XOUT], FP16, isOutput=True)

    with TileContext(nc) as tc:
        with (
            tc.tile_pool(name="const", bufs=1) as cpool,
            tc.tile_pool(name="io", bufs=1) as iopool,
            tc.tile_pool(name="tmp", bufs=4) as tmppool,
            tc.tile_pool(name="l2", bufs=3) as l2pool,
            tc.tile_pool(name="stp", bufs=3) as stpool,
            tc.tile_pool(name="psy", bufs=2, space="PSUM") as psyp,
            tc.tile_pool(name="psx", bufs=2, space="PSUM") as psxp,
        ):
            wyt = cpool.tile([128, YOUT], FP16)
            nc.sync.dma_start(out=wyt[:, :], in_=wy[:, :])
            wy0t = cpool.tile([128, n0 * YOUT], FP16)
            nc.sync.dma_start(out=wy0t[:, :], in_=wy0[:, :])
            wy0v = wy0t.rearrange("p (t y) -> p t y", t=n0)
            wy2t = cpool.tile([128, n2 * YOUT], FP16)
            nc.sync.dma_start(out=wy2t[:, :], in_=wy2[:, :])
            wy2v = wy2t.rearrange("p (t y) -> p t y", t=n2)
            wxt = cpool.tile([128, XOUT], FP16)
            nc.sync.dma_start(out=wxt[:, :], in_=wx[:, :])

            L0 = iopool.tile([128, B6 * ZI * XP], FP16)
            for g in range(2):
                nc.sync.dma_start(out=L0[64 * g:64 * g + 64, :],
                                  in_=v[64 * g:64 * g + 64, :])
            L0v = L0.rearrange("p (z b x) -> p z b x", z=ZI, b=B6)

            L1 = iopool.tile([128, B6 * ZSH * XP], FP16)
            L1v = L1.rearrange("p (k r b x) -> p k r b x", k=5, r=4, b=B6)

            groups = [(g, bp) for g in range(2) for bp in range(3)]

            xw = list(COPY_W)
            xacc = [0.0, 0.0]

            def pick():
                tot = sum(xw)
                for k in range(2):
                    xacc[k] += xw[k] / tot
                i = max(range(2), key=lambda k: xacc[k])
                xacc[i] -= 1.0
                return i

            def copy_ps(dst, src):
                if pick() == 0:
                    nc.vector.tensor_copy(out=dst, in_=src)
                else:
                    nc.scalar.copy(dst, src)

            def emit_z(gi):
                """z-pass phases r not PE-offloaded, for group index gi."""
                g, bp = groups[gi]
                lo, hi = 64 * g, 64 * g + YIN
                for r in range(4):
                    if r in PE_OFFLOAD[gi]:
                        continue
                    dst = L1v[lo:hi, :, r, 2 * bp:2 * bp + 2, 0:XIN]
                    taps = _ZTAPS[r]
                    t0, w0 = taps[0]
                    if Z_MODE[gi] == "hybrid" and len(taps) == 4:
                        # DVE scaled muls (4x mode) into contiguous tmps; gpsimd
                        # adds tmp pairs (contiguous (b,x) merges to 3D); DVE
                        # does the final strided add into L1.
                        tms = []
                        for t, w in taps:
                            tm = tmppool.tile([128, 2 * 5 * XIN], FP16)
                            tmv = tm.rearrange("p (k b x) -> p k b x", k=5, b=2)
                            nc.vector.tensor_scalar_mul(
                                tmv[lo:hi], L0v[lo:hi, t:t + 5, 2 * bp:2 * bp + 2, 0:XIN], w)
                            tms.append(tm)
                        pa = tmppool.tile([128, 2 * 5 * XIN], FP16)
                        pb = tmppool.tile([128, 2 * 5 * XIN], FP16)
                        nc.gpsimd.tensor_tensor(
                            out=pa[lo:hi, :], in0=tms[0][lo:hi, :],
                            in1=tms[1][lo:hi, :], op=ADD)
                        nc.gpsimd.tensor_tensor(
                            out=pb[lo:hi, :], in0=tms[2][lo:hi, :],
                            in1=tms[3][lo:hi, :], op=ADD)
                        nc.vector.tensor_tensor(
                            out=dst, in0=pa.rearrange("p (k b x) -> p k b x", k=5, b=2)[lo:hi],
                            in1=pb.rearrange("p (k b x) -> p k b x", k=5, b=2)[lo:hi], op=ADD)
                    else:
                        nc.vector.tensor_scalar_mul(
                            dst, L0v[lo:hi, t0:t0 + 5, 2 * bp:2 * bp + 2, 0:XIN], w0)
                        for t, w in taps[1:]:
                            tm = tmppool.tile([128, 2 * 5 * XIN], FP16)
                            tmv = tm.rearrange("p (k b x) -> p k b x", k=5, b=2)
                            nc.vector.tensor_scalar_mul(
                                tmv[lo:hi], L0v[lo:hi, t:t + 5, 2 * bp:2 * bp + 2, 0:XIN], w)
                            nc.vector.tensor_tensor(
                                out=dst, in0=dst, in1=tmv[lo:hi], op=ADD)

            def emit_y(gi):
                """y-pass for group: 5 psum quads of 4 zo each -> L2 tile."""
                g, bp = groups[gi]
                lo, hi = 64 * g, 64 * g + YIN
                L2g = l2pool.tile([128, M_TOT], FP16)
                offs = (0, 192, 512, 704)
                wyo = {0: wy0v, 2: wy2v}
                for q in range(5):
                    psy = psyp.tile([128, 1024], FP32)
                    for s in range(4):
                        zo, r, off = 4 * q + s, s, offs[s]
                        if r in PE_OFFLOAD[gi]:
                            taps = _ZTAPS[r]
                            for i, (t, w) in enumerate(taps):
                                nc.tensor.matmul(
                                    psy[:, off:off + YOUT],
                                    lhsT=L0v[lo:hi, q + t, 2 * bp:2 * bp + 2, :],
                                    rhs=wyo[r][lo:hi, i, :],
                                    start=(i == 0), stop=(i == len(taps) - 1),
                                )
                        else:
                            nc.tensor.matmul(
                                psy[:, off:off + YOUT],
                                lhsT=L1v[lo:hi, q, r, 2 * bp:2 * bp + 2, :],
                                rhs=wyt[lo:hi, :],
                                start=True, stop=True,
                            )
                    psyv = psy.rearrange("p (h x) -> p h x", h=2)
                    copy_ps(
                        L2g.rearrange("p (h x) -> p h x", h=10)[:, 2 * q:2 * q + 2, :],
                        psyv[:, :, 0:2 * YOUT])
                return L2g

            def emit_x(gi, L2g):
                """x-pass + staging + out DMA for the 2 bc of this group."""
                g, bp = groups[gi]
                L2j = L2g.rearrange("p (k j) -> p k j", j=NCH)
                for bm in range(2):
                    bc = 6 * g + 2 * bp + bm
                    st = stpool.tile([128, NCH * XOUT], FP16)
                    for pair in range(5):
                        psx = psxp.tile([128, 1024], FP32)
                        for u in range(6):
                            j = 6 * pair + u
                            off = 512 * (u // 3) + 160 * (u % 3)
                            nc.tensor.matmul(
                                psx[:, off:off + XOUT],
                                lhsT=L2j[64 * bm:64 * bm + XIN, :, j],
                                rhs=wxt[64 * bm:64 * bm + XIN, :],
                                start=True, stop=True,
                            )
                        psxv = psx.rearrange("p (h x) -> p h x", h=2)
                        dst = st.rearrange("p (pr x) -> p pr x", pr=5)[:, pair, :]
                        dstv = dst.rearrange("p (h x) -> p h x", h=2)
                        copy_ps(dstv, psxv[:, :, 0:480])
                        if pair == 1:
                            nc.sync.dma_start(
                                out=out[bc].rearrange("(p r) x -> p (r x)", p=128)[:, 0:1920],
                                in_=st[:, 0:1920])
                    nc.sync.dma_start(
                        out=out[bc].rearrange("(p r) x -> p (r x)", p=128)[:, 1920:4800],
                        in_=st[:, 1920:4800])

            # --- software-pipelined emission: x(k) before y(k+1) on PE ---
            emit_z(0)
            emit_z(1)
            L2s = {0: emit_y(0)}
            for k in range(6):
                if k + 2 < 6:
                    emit_z(k + 2)
                emit_x(k, L2s.pop(k))
                if k + 1 < 6:
                    L2s[k + 1] = emit_y(k + 1)
    nc.compile()
    return nc


def _get_nc():
    if "nc" not in _NC_CACHE:
        _NC_CACHE["nc"] = _build_nc()
    return _NC_CACHE["nc"]


def _host_weights():
    f16 = np.float16
    ey = _exp_mat(YIN, YOUT)
    ex = _exp_mat(XIN, XOUT)
    wy128 = np.zeros((128, YOUT), dtype=np.float32)
    wy128[0:YIN] = ey
    wy128[64:64 + YIN] = ey
    wx128 = np.zeros((128, XOUT), dtype=np.float32)
    wx128[0:XIN] = ex
    wx128[64:64 + XIN] = ex
    def scaled(r):
        taps = _ZTAPS[r]
        m = np.zeros((128, len(taps) * YOUT), dtype=np.float32)
        for i, (t, w) in enumerate(taps):
            m[:, i * YOUT:(i + 1) * YOUT] = wy128 * w
        return m.astype(f16)
    return wy128.astype(f16), scaled(0), scaled(2), wx128.astype(f16)


def kernel(v):
    from concourse.bass_utils import run_bass_kernel_spmd

    f16 = np.float16
    v = np.asarray(v).astype(np.float32).reshape(12, ZIN, YIN, XIN)
    wy_h, wy0_h, wy2_h, wx_h = _host_weights()

    in_maps = []
    for c in range(N_CORES):
        slab = v[:, 5 * c:5 * c + ZI]                      # [12, 8, 52, 44]
        arr = np.zeros((128, ZI, B6, XP), dtype=f16)
        arr[0:YIN, :, :, 0:XIN] = slab[0:6].transpose(2, 1, 0, 3)   # y, z, b, x
        arr[64:64 + YIN, :, :, 0:XIN] = slab[6:12].transpose(2, 1, 0, 3)
        in_maps.append({
            "v": np.ascontiguousarray(arr.reshape(128, B6 * ZI * XP)),
            "wy": wy_h, "wy0": wy0_h, "wy2": wy2_h, "wx": wx_h,
        })

    nc = _get_nc()
    res = run_bass_kernel_spmd(nc, in_maps, core_ids=list(range(N_CORES)))

    outf = np.empty((12, ZOUT, YOUT, XOUT), dtype=np.float32)
    for c in range(N_CORES):
        blk = res.results[c]["out"]                        # [12, 3840, 160] fp16
        outf[:, ZSH * c:ZSH * (c + 1)] = (
            blk.astype(np.float32).reshape(12, ZSH, YOUT, XOUT))
    return outf.reshape(4, 3, ZOUT, YOUT, XOUT)
